# revision 1
# baseline (speedup 1.0000x reference)
"""4-layer GCN (DglGCNNet) Trainium2 kernel, 8 NeuronCores.

Strategy (dst-partitioned graph; halo exchange == AllGather since the graph
is uniform random):
  - Host: bin-pack nodes into 8*98 blocks of <=128 dst nodes each, balancing
    per-block in-edge counts.  Core c owns 98 blocks (12544 padded node
    slots).  Edges are grouped by (dst block, src sub-table) and padded to
    128-edge tiles.
  - Device, per layer:
      A: h = X @ W per 128-node chunk on PE (X kept feat-major in SBUF,
         norm_src pre-folded into X rows), cast fp16, DMA to DRAM.
      B: AllGather h across the 8 cores -> fp16 table [100352, 128].
      C: per 2-block group: dma_gather edge messages from the table (one
         call per src sub-table of 32768 rows -- int16 index range); build
         one-hot indicator tiles from slot ids with a broadcast is_equal on
         DVE; segment-sum via indicator matmuls accumulating in PSUM;
         epilogue: *norm_dst + bias, tanh, *next-layer norm_src,
         PE-transpose back into the feat-major X buffer.
"""

import numpy as np

import concourse.bass as bass
import concourse.mybir as mybir
import concourse.tile as tile
from concourse import bacc

P = 128
D_IN = 128
D_HID = 128
D_OUT = 64
N_LAYERS = 4
G = 2  # dst blocks per gather-call group


class Cfg:
    def __init__(self, n_nodes, n_cores, blocks_per_core, subsz=32768):
        self.N = n_nodes
        self.NCORES = n_cores
        self.NBLK = blocks_per_core
        self.NP_CORE = blocks_per_core * P
        self.NPAD = n_cores * self.NP_CORE
        self.SUBSZ = subsz
        self.SUBS = list(range(0, self.NPAD, subsz))  # sub-table bases
        assert self.NPAD >= n_nodes
        assert self.NBLK % G == 0


FULL_CFG = Cfg(n_nodes=100000, n_cores=8, blocks_per_core=98)


# ---------------------------------------------------------------- host side


def degree_norms(edge_index, n):
    src = np.asarray(edge_index[0], dtype=np.int64)
    dst = np.asarray(edge_index[1], dtype=np.int64)
    out_deg = np.bincount(src, minlength=n).astype(np.float32)
    in_deg = np.bincount(dst, minlength=n).astype(np.float32)
    norm_src = np.where(out_deg > 0, 1.0 / np.sqrt(np.maximum(out_deg, 1.0)),
                        0.0).astype(np.float32)
    norm_dst = np.where(in_deg > 0, 1.0 / np.sqrt(np.maximum(in_deg, 1.0)),
                        0.0).astype(np.float32)
    return norm_src, norm_dst


def preprocess(edge_index, features, norms, cfg):
    """Partition the graph; build per-core device inputs.

    Returns (in_maps, kq, pos_of).
    """
    N, NCORES, NBLK, NP_CORE, NPAD = (
        cfg.N, cfg.NCORES, cfg.NBLK, cfg.NP_CORE, cfg.NPAD)
    norm_src, norm_dst = norms
    src = np.asarray(edge_index[0], dtype=np.int64)
    dst = np.asarray(edge_index[1], dtype=np.int64)
    in_deg = np.bincount(dst, minlength=N).astype(np.int64)

    # --- bin-pack nodes into NB blocks (<=P nodes each), balancing edges
    NB = NCORES * NBLK
    import heapq
    order = np.argsort(-in_deg, kind="stable")
    heap = [(0, b) for b in range(NB)]
    heapq.heapify(heap)
    counts = np.zeros(NB, np.int64)
    block_of = np.empty(N, np.int32)
    slot_of = np.empty(N, np.int32)
    for n in order:
        while True:
            w, b = heapq.heappop(heap)
            if counts[b] < P:
                break
        block_of[n] = b
        slot_of[n] = counts[b]
        counts[b] += 1
        heapq.heappush(heap, (w + int(in_deg[n]), b))

    block_w = np.bincount(block_of, weights=in_deg.astype(np.float64),
                          minlength=NB).astype(np.int64)

    # --- blocks -> cores (snake by weight to balance per-core edge totals)
    worder = np.argsort(-block_w, kind="stable")
    core_of_block = np.empty(NB, np.int32)
    idx_in_core = np.empty(NB, np.int32)
    fill = np.zeros(NCORES, np.int32)
    for i, b in enumerate(worder):
        rnd, j = divmod(i, NCORES)
        c = j if rnd % 2 == 0 else NCORES - 1 - j
        core_of_block[b] = c
        idx_in_core[b] = fill[c]
        fill[c] += 1

    pos_of = (core_of_block[block_of].astype(np.int64) * NP_CORE
              + idx_in_core[block_of].astype(np.int64) * P
              + slot_of.astype(np.int64))

    # --- per-(core, block, sub-table) edge counts -> caps
    NSUB = len(cfg.SUBS)
    pos_src = pos_of[src]
    q_of_edge = pos_src // cfg.SUBSZ
    e_blk = block_of[dst]
    e_core = core_of_block[e_blk]
    e_bic = idx_in_core[e_blk]

    cnt = np.zeros((NCORES, NBLK, NSUB), np.int64)
    np.add.at(cnt, (e_core, e_bic, q_of_edge), 1)
    kq = [int(-(-cnt[:, :, q].max() // P)) for q in range(NSUB)]
    kt = sum(kq)
    qoff_tiles = np.concatenate([[0], np.cumsum(kq)]).astype(np.int64)

    in_maps = []
    for c in range(NCORES):
        m = e_core == c
        bb = e_bic[m].astype(np.int64)
        qq = q_of_edge[m]
        ps = pos_src[m]
        sl = slot_of[dst[m]]
        # sort by (block, quadrant, src) for gather locality
        o = np.argsort((bb * NSUB + qq) * NPAD + ps, kind="stable")
        bb, qq, ps, sl = bb[o], qq[o], ps[o], sl[o]

        # per-(b, q) destination slot ranges within the padded edge stream
        seg = bb * NSUB + qq
        seg_cnt = np.bincount(seg, minlength=NBLK * NSUB).reshape(NBLK, NSUB)
        slots_q = np.array([k * P for k in kq], np.int64)
        seg_start = (np.arange(NBLK)[:, None] * (kt * P)
                     + np.concatenate([[0], np.cumsum(slots_q)])[:-1][None, :])
        starts_flat = seg_start.reshape(-1)
        cum = np.zeros(NBLK * NSUB, np.int64)
        cum[1:] = np.cumsum(seg_cnt.reshape(-1))[:-1]
        eslot = starts_flat[seg] + (np.arange(len(bb)) - cum[seg])

        # padded edge stream arrays (slot=255 kills padding in the indicator)
        tot = NBLK * kt * P
        idx16 = np.zeros(tot, np.int16)
        slotv = np.full(tot, 255.0, np.float16)
        idx16[eslot] = (ps - np.asarray(cfg.SUBS, np.int64)[qq]).astype(
            np.int16)
        slotv[eslot] = sl.astype(np.float16)

        # slot tensor [P, NBLK*KT]: tile t of block b <- edges [t*128, ...)
        slot_arr = np.ascontiguousarray(
            slotv.reshape(NBLK * kt, P).T)

        # gather index tensor, compact [16, COLS]; call (group, q) covers
        # G consecutive blocks' (b, q) segments concatenated
        ngrp = NBLK // G
        gw = G * kt * P // 16  # int16 cols per group
        gidxc = np.zeros((16, ngrp * gw), np.int16)
        stream = idx16.reshape(NBLK, kt * P)
        for g in range(ngrp):
            parts = []
            for q in range(NSUB):
                s0 = int(qoff_tiles[q]) * P
                s1 = int(qoff_tiles[q + 1]) * P
                for b01 in range(G):
                    parts.append(stream[g * G + b01, s0:s1])
            flat = np.concatenate(parts)
            j = np.arange(len(flat))
            a16 = np.zeros((16, len(flat) // 16), np.int16)
            a16[j % 16, j // 16] = flat
            gidxc[:, g * gw:(g + 1) * gw] = a16

        in_maps.append({"gidxc": gidxc, "slot": slot_arr})

    # --- node-order-dependent arrays
    xpad = np.zeros((NPAD, D_IN), np.float32)
    xpad[pos_of] = np.asarray(features, np.float32) * norm_src[:, None]
    nsrc_pad = np.zeros(NPAD, np.float32)
    nsrc_pad[pos_of] = norm_src
    ndst_pad = np.zeros(NPAD, np.float32)
    ndst_pad[pos_of] = norm_dst
    for c in range(NCORES):
        s = slice(c * NP_CORE, (c + 1) * NP_CORE)
        in_maps[c]["x0T"] = np.ascontiguousarray(xpad[s].T)
        in_maps[c]["nsrc"] = np.ascontiguousarray(
            nsrc_pad[s].reshape(NBLK, P).T)
        in_maps[c]["ndst"] = np.ascontiguousarray(
            ndst_pad[s].reshape(NBLK, P).T)

    return in_maps, kq, pos_of


def make_in_maps(inputs, cfg):
    norms = degree_norms(inputs["edge_index"], cfg.N)
    in_maps, kq, pos_of = preprocess(
        inputs["edge_index"], inputs["features"], norms, cfg)
    iota = np.tile(np.arange(P, dtype=np.float16), (P, 1))
    ident = np.eye(P, dtype=np.float32)
    for m in in_maps:
        m["iota"] = iota
        m["ident"] = ident
        for l in range(N_LAYERS):
            W = np.asarray(inputs[f"W{l}"], np.float32)
            b = np.asarray(inputs[f"b{l}"], np.float32)
            if W.shape[1] < D_IN:  # pad last layer to width 128
                W = np.pad(W, ((0, 0), (0, D_IN - W.shape[1])))
                b = np.pad(b, (0, D_IN - b.shape[0]))
            m[f"W{l}"] = W
            m[f"bb{l}"] = np.ascontiguousarray(
                np.broadcast_to(b, (P, D_IN)))
    return in_maps, kq, pos_of


def assemble_output(results, pos_of, cfg):
    full = np.concatenate([r["y"] for r in results], axis=0)
    return np.ascontiguousarray(full[pos_of])


# -------------------------------------------------------------- device side


def build_nc(cfg, kq):
    NCORES, NBLK, NP_CORE, NPAD = cfg.NCORES, cfg.NBLK, cfg.NP_CORE, cfg.NPAD
    NSUB = len(cfg.SUBS)
    assert NSUB == len(kq)
    kt = sum(kq)
    ngrp = NBLK // G
    gw = G * kt * P // 16
    T = NBLK * kt
    D = D_IN
    f32, f16, i16 = mybir.dt.float32, mybir.dt.float16, mybir.dt.int16
    qoff_tiles = np.concatenate([[0], np.cumsum(kq)]).astype(int)

    nc = bacc.Bacc("TRN2", target_bir_lowering=False, debug=False,
                   num_devices=NCORES)

    x0T_d = nc.dram_tensor("x0T", [D, NP_CORE], f32, kind="ExternalInput")
    gidxc_d = nc.dram_tensor("gidxc", [16, ngrp * gw], i16,
                             kind="ExternalInput")
    slot_d = nc.dram_tensor("slot", [P, T], f16, kind="ExternalInput")
    nsrc_d = nc.dram_tensor("nsrc", [P, NBLK], f32, kind="ExternalInput")
    ndst_d = nc.dram_tensor("ndst", [P, NBLK], f32, kind="ExternalInput")
    iota_d = nc.dram_tensor("iota", [P, P], f16, kind="ExternalInput")
    ident_d = nc.dram_tensor("ident", [P, P], f32, kind="ExternalInput")
    W_d = [nc.dram_tensor(f"W{l}", [D, D], f32, kind="ExternalInput")
           for l in range(N_LAYERS)]
    B_d = [nc.dram_tensor(f"bb{l}", [P, D], f32, kind="ExternalInput")
           for l in range(N_LAYERS)]
    y_d = nc.dram_tensor("y", [NP_CORE, D_OUT], f32, kind="ExternalOutput")

    hloc = [nc.dram_tensor(f"hloc{i}", [NP_CORE, D], f16) for i in range(2)]
    hful = [nc.dram_tensor(f"hful{i}", [NPAD, D], f16, addr_space="Shared")
            for i in range(2)]

    # persistent SBUF
    xT = [nc.alloc_sbuf_tensor(f"xT{i}", [D, NP_CORE], f32).ap()
          for i in range(2)]
    slot_s = nc.alloc_sbuf_tensor("slot_s", [P, T], f16).ap()
    nsrc_s = nc.alloc_sbuf_tensor("nsrc_s", [P, NBLK], f32).ap()
    ndst_s = nc.alloc_sbuf_tensor("ndst_s", [P, NBLK], f32).ap()
    iota_s = nc.alloc_sbuf_tensor("iota_s", [P, P], f16).ap()
    ident_s = nc.alloc_sbuf_tensor("ident_s", [P, P], f32).ap()
    W_s = [nc.alloc_sbuf_tensor(f"W_s{l}", [D, D], f32).ap()
           for l in range(N_LAYERS)]
    B_s = [nc.alloc_sbuf_tensor(f"B_s{l}", [P, D], f32).ap()
           for l in range(N_LAYERS)]

    rg = [list(range(NCORES))]

    def bcast16(dram, col0, w):
        """AP reading [16, w] at col0 replicated 8x -> [128, w]."""
        a = dram[:, col0:col0 + w]
        return bass.AP(a.tensor, a.offset, [[0, 8]] + list(a.ap))

    with tile.TileContext(nc) as tc:
        with (
            tc.tile_pool(name="gip", bufs=3) as gip,
            tc.tile_pool(name="msgp", bufs=2) as msgp,
            tc.tile_pool(name="indp", bufs=3) as indp,
            tc.tile_pool(name="hap", bufs=4) as hap,
            tc.tile_pool(name="epp", bufs=4) as epp,
            tc.tile_pool(name="psA", bufs=2, space="PSUM") as psA,
            tc.tile_pool(name="psC", bufs=2, space="PSUM") as psC,
            tc.tile_pool(name="psT", bufs=2, space="PSUM") as psT,
        ):
            # ---- load constants
            nc.sync.dma_start(out=xT[0], in_=x0T_d[:, :])
            nc.sync.dma_start(out=slot_s, in_=slot_d[:, :])
            nc.sync.dma_start(out=nsrc_s, in_=nsrc_d[:, :])
            nc.sync.dma_start(out=ndst_s, in_=ndst_d[:, :])
            nc.sync.dma_start(out=iota_s, in_=iota_d[:, :])
            nc.sync.dma_start(out=ident_s, in_=ident_d[:, :])
            for l in range(N_LAYERS):
                nc.sync.dma_start(out=W_s[l], in_=W_d[l][:, :])
                nc.sync.dma_start(out=B_s[l], in_=B_d[l][:, :])

            for l in range(N_LAYERS):
                last = l == N_LAYERS - 1
                xcur = xT[l % 2]
                xnext = xT[(l + 1) % 2]
                hl = hloc[l % 2]
                hf = hful[l % 2]

                # ---- A: h = X @ W (node-major chunks), cast fp16, to DRAM
                for b in range(NBLK):
                    ph = psA.tile([P, D], f32, tag="psA")
                    nc.tensor.matmul(ph[:], lhsT=xcur[:, b * P:(b + 1) * P],
                                     rhs=W_s[l][:, :], start=True, stop=True)
                    hsb = hap.tile([P, D], f16, tag="h")
                    nc.vector.tensor_copy(out=hsb[:], in_=ph[:])
                    nc.sync.dma_start(out=hl[b * P:(b + 1) * P, :],
                                      in_=hsb[:])

                # ---- B: AllGather
                nc.gpsimd.collective_compute(
                    "AllGather", mybir.AluOpType.bypass, replica_groups=rg,
                    ins=[hl[:, :]], outs=[hf[:, :]])

                # ---- C: gather + segment-sum + epilogue per 2-block group
                for g in range(ngrp):
                    gi = gip.tile([P, gw], i16, tag="gi")
                    nc.sync.dma_start(out=gi[:],
                                      in_=bcast16(gidxc_d, g * gw, gw))
                    msg = msgp.tile([P, G * kt * D], f16, tag="msg")
                    coff = 0  # int16 col offset into gi
                    moff = 0  # tile offset into msg
                    for q in range(NSUB):
                        if kq[q] == 0:
                            continue
                        nidx = G * kq[q] * P
                        sub = hf[cfg.SUBS[q]:
                                 min(cfg.SUBS[q] + cfg.SUBSZ, NPAD), :]
                        nc.gpsimd.dma_gather(
                            out_ap=msg[:, moff * D:(moff + G * kq[q]) * D]
                            .rearrange("p (t e) -> p t e", e=D),
                            in_ap=sub,
                            idxs_ap=gi[:, coff:coff + nidx // 16],
                            num_idxs=nidx,
                            num_idxs_reg=nidx,
                            elem_size=D,
                            single_packet=False)
                        coff += nidx // 16
                        moff += G * kq[q]

                    for b01 in range(G):
                        b = g * G + b01
                        ind = indp.tile([P, kt * P], f16, tag="ind")
                        ind_ap = ind[:]
                        ind3 = bass.AP(ind_ap.tensor, ind_ap.offset,
                                       [[kt * P, P], [P, kt], [1, P]])
                        slot3 = slot_s[:, b * kt:(b + 1) * kt].to_broadcast(
                            [P, kt, P])
                        iota3 = bass.AP(iota_s.tensor, iota_s.offset,
                                        [[P, P], [0, kt], [1, P]])
                        nc.vector.tensor_tensor(
                            out=ind3, in0=slot3, in1=iota3,
                            op=mybir.AluOpType.is_equal)

                        pagg = psC.tile([P, D], f32, tag="psC")
                        for t in range(kt):
                            q = int(np.searchsorted(qoff_tiles, t,
                                                    side="right")) - 1
                            j = t - int(qoff_tiles[q])
                            mcol = (G * int(qoff_tiles[q])
                                    + b01 * kq[q] + j)
                            nc.tensor.matmul(
                                pagg[:],
                                lhsT=ind[:, t * P:(t + 1) * P],
                                rhs=msg[:, mcol * D:(mcol + 1) * D],
                                start=(t == 0), stop=(t == kt - 1))

                        t1 = epp.tile([P, D], f32, tag="t1")
                        nc.vector.tensor_scalar(
                            out=t1[:], in0=pagg[:],
                            scalar1=ndst_s[:, b:b + 1], scalar2=None,
                            op0=mybir.AluOpType.mult)
                        nc.vector.tensor_add(out=t1[:], in0=t1[:],
                                             in1=B_s[l][:, :])
                        if last:
                            nc.sync.dma_start(
                                out=y_d[b * P:(b + 1) * P, :],
                                in_=t1[:, :D_OUT])
                        else:
                            t2 = epp.tile([P, D], f32, tag="t2")
                            nc.scalar.activation(
                                out=t2[:], in_=t1[:],
                                func=mybir.ActivationFunctionType.Tanh)
                            nc.vector.tensor_scalar(
                                out=t2[:], in0=t2[:],
                                scalar1=nsrc_s[:, b:b + 1], scalar2=None,
                                op0=mybir.AluOpType.mult)
                            pt = psT.tile([P, P], f32, tag="psT")
                            nc.tensor.transpose(pt[:], t2[:], ident_s)
                            nc.vector.tensor_copy(
                                out=xnext[:, b * P:(b + 1) * P], in_=pt[:])

    nc.compile()
    return nc


_CACHE = {}
LAST_EXEC_NS = None


def kernel(**inputs):
    global LAST_EXEC_NS
    from concourse.bass_utils import run_bass_kernel_spmd

    cfg = FULL_CFG
    in_maps, kq, pos_of = make_in_maps(inputs, cfg)
    key = ("full", tuple(kq))
    if key not in _CACHE:
        _CACHE[key] = build_nc(cfg, kq)
    nc = _CACHE[key]
    res = run_bass_kernel_spmd(nc, in_maps, list(range(cfg.NCORES)))
    LAST_EXEC_NS = res.exec_time_ns
    out = assemble_output(res.results, pos_of, cfg)
    return out.astype(np.float32)



# revision 2
# speedup vs baseline: 486.0629x; 486.0629x over previous
"""4-layer GCN (DglGCNNet) Trainium2 kernel, 8 NeuronCores.

Strategy (dst-partitioned graph; halo exchange == AllGather into a Shared
DRAM table, since the graph is uniform random):
  - Host: bin-pack nodes into 8*98 blocks of <=128 dst nodes each, balancing
    per-block in-edge counts.  Core c owns 98 blocks.  Edges are grouped by
    (dst block, src sub-table) and padded to 128-edge tiles.
  - Device, per layer:
      h = X @ W computed inline in the previous layer's epilogue (fp16);
      one batched DMA per 7-block group writes h to DRAM; AllGather into a
      Shared fp16 table [100352, 128]; per 7-block group: dma_gather edge
      messages (one call per 32768-row src sub-table -- int16 index range,
      calls spread over 4 SWDGE queues greedily by load); one-hot
      indicator tiles from slot ids via broadcast is_equal on DVE;
      segment-sum via indicator matmuls accumulated in PSUM; epilogue
      *norm_dst + bias, tanh, *norm_src folded ahead of the next matmul.

Perf notes (measured on this 8-core axon setup): the dma_gather is ~90%
of device time and is DMA-queue-bound (~27 GB/s per SWDGE queue); 4
queues + few large calls (~15 us fixed cost per call) is the optimum
found.  AllGather into the Shared table is nearly free.
"""

import numpy as np

import concourse.bass as bass
import concourse.mybir as mybir
import concourse.tile as tile
from concourse import bacc

P = 128
D_IN = 128
D_HID = 128
D_OUT = 64
N_LAYERS = 4


class Cfg:
    def __init__(self, n_nodes, n_cores, blocks_per_core, g=7, subsz=32768):
        self.N = n_nodes
        self.NCORES = n_cores
        self.NBLK = blocks_per_core
        self.NP_CORE = blocks_per_core * P
        self.NPAD = n_cores * self.NP_CORE
        self.SUBSZ = subsz
        self.SUBS = list(range(0, self.NPAD, subsz))
        self.G = g
        assert self.NPAD >= n_nodes
        assert self.NBLK % g == 0


FULL_CFG = Cfg(n_nodes=100000, n_cores=8, blocks_per_core=98)


# ---------------------------------------------------------------- host side


def degree_norms(edge_index, n):
    src = np.asarray(edge_index[0], dtype=np.int64)
    dst = np.asarray(edge_index[1], dtype=np.int64)
    out_deg = np.bincount(src, minlength=n).astype(np.float32)
    in_deg = np.bincount(dst, minlength=n).astype(np.float32)
    norm_src = np.where(out_deg > 0, 1.0 / np.sqrt(np.maximum(out_deg, 1.0)),
                        0.0).astype(np.float32)
    norm_dst = np.where(in_deg > 0, 1.0 / np.sqrt(np.maximum(in_deg, 1.0)),
                        0.0).astype(np.float32)
    return norm_src, norm_dst


def preprocess(edge_index, features, norms, cfg):
    N, NCORES, NBLK, NP_CORE, NPAD, G = (
        cfg.N, cfg.NCORES, cfg.NBLK, cfg.NP_CORE, cfg.NPAD, cfg.G)
    norm_src, norm_dst = norms
    src = np.asarray(edge_index[0], dtype=np.int64)
    dst = np.asarray(edge_index[1], dtype=np.int64)
    in_deg = np.bincount(dst, minlength=N).astype(np.int64)

    # --- bin-pack nodes into NB blocks (<=P nodes each), balancing edges
    NB = NCORES * NBLK
    import heapq
    order = np.argsort(-in_deg, kind="stable")
    heap = [(0, b) for b in range(NB)]
    heapq.heapify(heap)
    counts = np.zeros(NB, np.int64)
    block_of = np.empty(N, np.int32)
    slot_of = np.empty(N, np.int32)
    for n in order:
        while True:
            w, b = heapq.heappop(heap)
            if counts[b] < P:
                break
        block_of[n] = b
        slot_of[n] = counts[b]
        counts[b] += 1
        heapq.heappush(heap, (w + int(in_deg[n]), b))

    block_w = np.bincount(block_of, weights=in_deg.astype(np.float64),
                          minlength=NB).astype(np.int64)

    # --- blocks -> cores (snake by weight to balance per-core edge totals)
    worder = np.argsort(-block_w, kind="stable")
    core_of_block = np.empty(NB, np.int32)
    idx_in_core = np.empty(NB, np.int32)
    fill = np.zeros(NCORES, np.int32)
    for i, b in enumerate(worder):
        rnd, j = divmod(i, NCORES)
        c = j if rnd % 2 == 0 else NCORES - 1 - j
        core_of_block[b] = c
        idx_in_core[b] = fill[c]
        fill[c] += 1

    pos_of = (core_of_block[block_of].astype(np.int64) * NP_CORE
              + idx_in_core[block_of].astype(np.int64) * P
              + slot_of.astype(np.int64))

    # --- per-(core, block, sub-table) edge counts -> caps
    NSUB = len(cfg.SUBS)
    pos_src = pos_of[src]
    q_of_edge = pos_src // cfg.SUBSZ
    e_blk = block_of[dst]
    e_core = core_of_block[e_blk]
    e_bic = idx_in_core[e_blk]

    cnt = np.zeros((NCORES, NBLK, NSUB), np.int64)
    np.add.at(cnt, (e_core, e_bic, q_of_edge), 1)
    kq = [int(-(-cnt[:, :, q].max() // P)) for q in range(NSUB)]
    kt = sum(kq)
    qoff_tiles = np.concatenate([[0], np.cumsum(kq)]).astype(np.int64)

    in_maps = []
    for c in range(NCORES):
        m = e_core == c
        bb = e_bic[m].astype(np.int64)
        qq = q_of_edge[m]
        ps = pos_src[m]
        sl = slot_of[dst[m]]
        # sort by (block, quadrant, src) for gather locality
        o = np.argsort((bb * NSUB + qq) * NPAD + ps, kind="stable")
        bb, qq, ps, sl = bb[o], qq[o], ps[o], sl[o]

        # per-(b, q) destination slot ranges within the padded edge stream
        seg = bb * NSUB + qq
        seg_cnt = np.bincount(seg, minlength=NBLK * NSUB).reshape(NBLK, NSUB)
        slots_q = np.array([k * P for k in kq], np.int64)
        seg_start = (np.arange(NBLK)[:, None] * (kt * P)
                     + np.concatenate([[0], np.cumsum(slots_q)])[:-1][None, :])
        starts_flat = seg_start.reshape(-1)
        cum = np.zeros(NBLK * NSUB, np.int64)
        cum[1:] = np.cumsum(seg_cnt.reshape(-1))[:-1]
        eslot = starts_flat[seg] + (np.arange(len(bb)) - cum[seg])

        # padded edge stream arrays (slot=255 kills padding in the indicator)
        tot = NBLK * kt * P
        idx16 = np.zeros(tot, np.int16)
        slotv = np.full(tot, 255.0, np.float16)
        idx16[eslot] = (ps - np.asarray(cfg.SUBS, np.int64)[qq]).astype(
            np.int16)
        slotv[eslot] = sl.astype(np.float16)

        # slot tensor [P, NBLK*KT]: tile t of block b <- edges [t*128, ...)
        slot_arr = np.ascontiguousarray(
            slotv.reshape(NBLK * kt, P).T)

        # gather index tensor, compact [16, COLS]; call (group, q) covers
        # G consecutive blocks' (b, q) segments concatenated
        ngrp = NBLK // G
        gw = G * kt * P // 16  # int16 cols per group
        gidxc = np.zeros((16, ngrp * gw), np.int16)
        stream = idx16.reshape(NBLK, kt * P)
        for g in range(ngrp):
            parts = []
            for q in range(NSUB):
                s0 = int(qoff_tiles[q]) * P
                s1 = int(qoff_tiles[q + 1]) * P
                for b01 in range(G):
                    parts.append(stream[g * G + b01, s0:s1])
            flat = np.concatenate(parts)
            j = np.arange(len(flat))
            a16 = np.zeros((16, len(flat) // 16), np.int16)
            a16[j % 16, j // 16] = flat
            gidxc[:, g * gw:(g + 1) * gw] = a16

        in_maps.append({"gidxc": gidxc, "slot": slot_arr})

    # --- node-order-dependent arrays
    xpad = np.zeros((NPAD, D_IN), np.float32)
    xpad[pos_of] = np.asarray(features, np.float32) * norm_src[:, None]
    nsrc_pad = np.zeros(NPAD, np.float32)
    nsrc_pad[pos_of] = norm_src
    ndst_pad = np.zeros(NPAD, np.float32)
    ndst_pad[pos_of] = norm_dst
    for c in range(NCORES):
        s = slice(c * NP_CORE, (c + 1) * NP_CORE)
        in_maps[c]["x0T"] = np.ascontiguousarray(xpad[s].T).astype(np.float16)
        in_maps[c]["nsrc"] = np.ascontiguousarray(
            nsrc_pad[s].reshape(NBLK, P).T)
        in_maps[c]["ndst"] = np.ascontiguousarray(
            ndst_pad[s].reshape(NBLK, P).T)

    return in_maps, kq, pos_of


def make_in_maps(inputs, cfg):
    norms = degree_norms(inputs["edge_index"], cfg.N)
    in_maps, kq, pos_of = preprocess(
        inputs["edge_index"], inputs["features"], norms, cfg)
    iota = np.tile(np.arange(P, dtype=np.float16), (P, 1))
    ident = np.eye(P, dtype=np.float32)
    for m in in_maps:
        m["iota"] = iota
        m["ident"] = ident
        for l in range(N_LAYERS):
            W = np.asarray(inputs[f"W{l}"], np.float32)
            b = np.asarray(inputs[f"b{l}"], np.float32)
            if W.shape[1] < D_IN:  # pad last layer to width 128
                W = np.pad(W, ((0, 0), (0, D_IN - W.shape[1])))
                b = np.pad(b, (0, D_IN - b.shape[0]))
            m[f"W{l}"] = W.astype(np.float16)
            m[f"bb{l}"] = np.ascontiguousarray(
                np.broadcast_to(b, (P, D_IN)))
    return in_maps, kq, pos_of


def assemble_output(results, pos_of, cfg):
    full = np.concatenate([r["y"] for r in results], axis=0)
    return np.ascontiguousarray(full[pos_of])


# -------------------------------------------------------------- device side


def build_nc(cfg, kq, nqueues=4):
    NCORES, NBLK, NP_CORE, NPAD, G = (
        cfg.NCORES, cfg.NBLK, cfg.NP_CORE, cfg.NPAD, cfg.G)
    NSUB = len(cfg.SUBS)
    assert NSUB == len(kq)
    kt = sum(kq)
    ngrp = NBLK // G
    gw = G * kt * P // 16
    T = NBLK * kt
    D = D_IN
    f32, f16, i16 = mybir.dt.float32, mybir.dt.float16, mybir.dt.int16
    qoff_tiles = np.concatenate([[0], np.cumsum(kq)]).astype(int)

    nc = bacc.Bacc("TRN2", target_bir_lowering=False, debug=False,
                   num_devices=NCORES, num_swdge_queues=nqueues)

    x0T_d = nc.dram_tensor("x0T", [D, NP_CORE], f16, kind="ExternalInput")
    gidxc_d = nc.dram_tensor("gidxc", [16, ngrp * gw], i16,
                             kind="ExternalInput")
    slot_d = nc.dram_tensor("slot", [P, T], f16, kind="ExternalInput")
    nsrc_d = nc.dram_tensor("nsrc", [P, NBLK], f32, kind="ExternalInput")
    ndst_d = nc.dram_tensor("ndst", [P, NBLK], f32, kind="ExternalInput")
    iota_d = nc.dram_tensor("iota", [P, P], f16, kind="ExternalInput")
    ident_d = nc.dram_tensor("ident", [P, P], f32, kind="ExternalInput")
    W_d = [nc.dram_tensor(f"W{l}", [D, D], f16, kind="ExternalInput")
           for l in range(N_LAYERS)]
    B_d = [nc.dram_tensor(f"bb{l}", [P, D], f32, kind="ExternalInput")
           for l in range(N_LAYERS)]
    y_d = nc.dram_tensor("y", [NP_CORE, D_OUT], f32, kind="ExternalOutput")

    hloc = [nc.dram_tensor(f"hloc{i}", [NP_CORE, D], f16) for i in range(2)]
    hful = [nc.dram_tensor(f"hful{i}", [NPAD, D], f16, addr_space="Shared")
            for i in range(2)]

    # persistent SBUF
    x0T_s = nc.alloc_sbuf_tensor("x0T_s", [D, NP_CORE], f16).ap()
    gidx_s = nc.alloc_sbuf_tensor("gidx_s", [P, ngrp * gw], i16).ap()
    slot_s = nc.alloc_sbuf_tensor("slot_s", [P, T], f16).ap()
    nsrc_s = nc.alloc_sbuf_tensor("nsrc_s", [P, NBLK], f32).ap()
    ndst_s = nc.alloc_sbuf_tensor("ndst_s", [P, NBLK], f32).ap()
    iota_s = nc.alloc_sbuf_tensor("iota_s", [P, P], f16).ap()
    ident_s = nc.alloc_sbuf_tensor("ident_s", [P, P], f32).ap()
    W_s = [nc.alloc_sbuf_tensor(f"W_s{l}", [D, D], f16).ap()
           for l in range(N_LAYERS)]
    B_s = [nc.alloc_sbuf_tensor(f"B_s{l}", [P, D], f32).ap()
           for l in range(N_LAYERS)]

    rg = [list(range(NCORES))]

    def hrows(dram, g, width):
        """[p, b01, f] view of dram rows [g*G*P, (g+1)*G*P) with row len
        `width`: partition p strides by width, b01 by P*width."""
        a = dram[g * G * P:(g + 1) * G * P, :]
        return bass.AP(a.tensor, a.offset,
                       [[width, P], [P * width, G], [1, width]])

    with tile.TileContext(nc) as tc:
        with (
            tc.tile_pool(name="msgp", bufs=2) as msgp,
            tc.tile_pool(name="indp", bufs=3) as indp,
            tc.tile_pool(name="xcp", bufs=3) as xcp,
            tc.tile_pool(name="epp", bufs=4) as epp,
            tc.tile_pool(name="hbp", bufs=2) as hbp,
            tc.tile_pool(name="psA", bufs=2, space="PSUM") as psA,
            tc.tile_pool(name="psC", bufs=2, space="PSUM") as psC,
            tc.tile_pool(name="psT", bufs=2, space="PSUM") as psT,
        ):
            # ---- load constants
            nc.sync.dma_start(out=x0T_s, in_=x0T_d[:, :])
            ga = gidxc_d[:, :]
            nc.sync.dma_start(out=gidx_s,
                              in_=bass.AP(ga.tensor, ga.offset,
                                          [[0, 8]] + list(ga.ap)))
            nc.sync.dma_start(out=slot_s, in_=slot_d[:, :])
            nc.sync.dma_start(out=nsrc_s, in_=nsrc_d[:, :])
            nc.sync.dma_start(out=ndst_s, in_=ndst_d[:, :])
            nc.sync.dma_start(out=iota_s, in_=iota_d[:, :])
            nc.sync.dma_start(out=ident_s, in_=ident_d[:, :])
            for l in range(N_LAYERS):
                nc.sync.dma_start(out=W_s[l], in_=W_d[l][:, :])
                nc.sync.dma_start(out=B_s[l], in_=B_d[l][:, :])

            # ---- layer 0 standalone A phase: h0 = X @ W0
            for g in range(ngrp):
                hblk = hbp.tile([P, G * D], f16, tag="hblk")
                for b01 in range(G):
                    b = g * G + b01
                    ph = psA.tile([P, D], f32, tag="psA")
                    nc.tensor.matmul(ph[:], lhsT=x0T_s[:, b * P:(b + 1) * P],
                                     rhs=W_s[0][:, :], start=True, stop=True)
                    nc.vector.tensor_copy(out=hblk[:, b01 * D:(b01 + 1) * D],
                                          in_=ph[:])
                nc.sync.dma_start(out=hrows(hloc[0], g, D),
                                  in_=hblk[:].rearrange("p (g f) -> p g f",
                                                        f=D))

            qload = [0] * nqueues

            def next_queue(n):
                qn = int(np.argmin(qload))
                qload[qn] += n
                return qn

            for l in range(N_LAYERS):
                last = l == N_LAYERS - 1
                hl = hloc[l % 2]
                hf = hful[l % 2]
                hnext = hloc[(l + 1) % 2]

                # ---- AllGather (into the Shared table: each core only
                #      ships its own 12544-row shard)
                nc.gpsimd.collective_compute(
                    "AllGather", mybir.AluOpType.bypass,
                    replica_groups=rg, ins=[hl[:, :]], outs=[hf[:, :]])

                # ---- per group: gather + segment-sum + epilogue (+ next
                #      layer's X @ W inline)
                for g in range(ngrp):
                    msg = msgp.tile([P, G * kt * D], f16, tag="msg")
                    coff = 0
                    moff = 0
                    for q in range(NSUB):
                        if kq[q] == 0:
                            continue
                        nidx = G * kq[q] * P
                        sub = hf[cfg.SUBS[q]:
                                 min(cfg.SUBS[q] + cfg.SUBSZ, NPAD), :]
                        nc.gpsimd.dma_gather(
                            out_ap=msg[:, moff * D:(moff + G * kq[q]) * D]
                            .rearrange("p (t e) -> p t e", e=D),
                            in_ap=sub,
                            idxs_ap=gidx_s[:, g * gw + coff:
                                           g * gw + coff + nidx // 16],
                            num_idxs=nidx,
                            num_idxs_reg=nidx,
                            elem_size=D,
                            single_packet=False,
                            queue_num=next_queue(nidx))
                        coff += nidx // 16
                        moff += G * kq[q]

                    hblk = yblk = None
                    if last:
                        yblk = hbp.tile([P, G * D_OUT], f32, tag="yblk")
                    else:
                        hblk = hbp.tile([P, G * D], f16, tag="hblk")

                    for b01 in range(G):
                        b = g * G + b01
                        ind = indp.tile([P, kt * P], f16, tag="ind")
                        ind_ap = ind[:]
                        ind3 = bass.AP(ind_ap.tensor, ind_ap.offset,
                                       [[kt * P, P], [P, kt], [1, P]])
                        slot3 = slot_s[:, b * kt:(b + 1) * kt].to_broadcast(
                            [P, kt, P])
                        iota3 = bass.AP(iota_s.tensor, iota_s.offset,
                                        [[P, P], [0, kt], [1, P]])
                        nc.vector.tensor_tensor(
                            out=ind3, in0=slot3, in1=iota3,
                            op=mybir.AluOpType.is_equal)

                        pagg = psC.tile([P, D], f32, tag="psC")
                        for t in range(kt):
                            q = int(np.searchsorted(qoff_tiles, t,
                                                    side="right")) - 1
                            j = t - int(qoff_tiles[q])
                            mcol = (G * int(qoff_tiles[q])
                                    + b01 * kq[q] + j)
                            nc.tensor.matmul(
                                pagg[:],
                                lhsT=ind[:, t * P:(t + 1) * P],
                                rhs=msg[:, mcol * D:(mcol + 1) * D],
                                start=(t == 0), stop=(t == kt - 1))

                        if last:
                            ysl = yblk[:, b01 * D_OUT:(b01 + 1) * D_OUT]
                            nc.vector.tensor_scalar(
                                out=ysl, in0=pagg[:, :D_OUT],
                                scalar1=ndst_s[:, b:b + 1], scalar2=None,
                                op0=mybir.AluOpType.mult)
                            nc.vector.tensor_add(
                                out=ysl, in0=ysl, in1=B_s[l][:, :D_OUT])
                        else:
                            t1 = epp.tile([P, D], f32, tag="t1")
                            nc.vector.tensor_scalar(
                                out=t1[:], in0=pagg[:],
                                scalar1=ndst_s[:, b:b + 1], scalar2=None,
                                op0=mybir.AluOpType.mult)
                            nc.vector.tensor_add(out=t1[:], in0=t1[:],
                                                 in1=B_s[l][:, :])
                            t2 = epp.tile([P, D], f32, tag="t2")
                            nc.scalar.activation(
                                out=t2[:], in_=t1[:],
                                func=mybir.ActivationFunctionType.Tanh)
                            nc.vector.tensor_scalar(
                                out=t2[:], in0=t2[:],
                                scalar1=nsrc_s[:, b:b + 1], scalar2=None,
                                op0=mybir.AluOpType.mult)
                            pt = psT.tile([P, P], f32, tag="psT")
                            nc.tensor.transpose(pt[:], t2[:], ident_s)
                            xcol = xcp.tile([P, P], f16, tag="xcol")
                            nc.vector.tensor_copy(out=xcol[:], in_=pt[:])
                            ph = psA.tile([P, D], f32, tag="psA")
                            nc.tensor.matmul(ph[:], lhsT=xcol[:],
                                             rhs=W_s[l + 1][:, :],
                                             start=True, stop=True)
                            nc.vector.tensor_copy(
                                out=hblk[:, b01 * D:(b01 + 1) * D],
                                in_=ph[:])

                    if last:
                        nc.sync.dma_start(
                            out=hrows(y_d, g, D_OUT),
                            in_=yblk[:].rearrange("p (g f) -> p g f",
                                                  f=D_OUT))
                    else:
                        nc.sync.dma_start(
                            out=hrows(hnext, g, D),
                            in_=hblk[:].rearrange("p (g f) -> p g f", f=D))

    nc.compile()
    return nc


_CACHE = {}


def kernel(**inputs):
    from concourse.bass_utils import run_bass_kernel_spmd

    cfg = FULL_CFG
    in_maps, kq, pos_of = make_in_maps(inputs, cfg)
    key = ("full", tuple(kq))
    if key not in _CACHE:
        _CACHE[key] = build_nc(cfg, kq)
    nc = _CACHE[key]
    res = run_bass_kernel_spmd(nc, in_maps, list(range(cfg.NCORES)))
    out = assemble_output(res.results, pos_of, cfg)
    return out.astype(np.float32)
